# revision 4
# baseline (speedup 1.0000x reference)
"""Switched-FC MoE kernel for Trainium2 (8 NeuronCores, data-parallel).

Math (per token b, expert e = y_index[b]):
    r = relu(x[b])
    h = relu(r @ W1[e] + b1[e])
    o = h @ W2[e] + b2[e]
    out[b] = x[b] + o * z[b]

Strategy:
  * Host: sort tokens by expert; pad each expert's token list to a multiple
    of 8 so every core gets an identical per-expert token count c_e.  This
    makes ONE Bass program (static per-block expert ids baked in at trace
    time) valid for all 8 cores.
  * Host: pre-relu + permute + transpose x so the device reads [D, C] tiles
    with the contraction dim on SBUF partitions (no on-device transpose).
  * Device: weights resident in SBUF; per token block (<=512 tokens, single
    expert): h^T = relu(W1[e]^T @ r^T + b1[e]); o^T = W2[e]^T @ h^T + b2[e].
    Output o^T streamed back to HBM.
  * Host: unpermute + out = x + z * o.
"""

import numpy as np

N_CORES = 8
TBLK = 512  # max token block (matmul moving free dim, fp32)

_PROGRAM_CACHE = {}


def _get_program(blocks, C, D, S, E, loop_n=1):
    """Build (or fetch cached) compiled Bass program for a block structure.

    loop_n > 1 wraps the token-block loop in an on-device For_i so the kernel
    body runs loop_n times back-to-back — used only for timing (amortizes the
    dispatch overhead out of the measurement).
    """
    key = (tuple(blocks), C, D, S, E, loop_n)
    if key in _PROGRAM_CACHE:
        return _PROGRAM_CACHE[key]

    import sys
    if "/opt/trn_rl_repo" not in sys.path:
        sys.path.insert(0, "/opt/trn_rl_repo")
    from contextlib import ExitStack

    import concourse.tile as tile
    from concourse import bacc, mybir

    DCH = D // 128  # number of 128-partition chunks of the model dim

    f32 = mybir.dt.float32
    Relu = mybir.ActivationFunctionType.Relu
    nc = bacc.Bacc("TRN2", target_bir_lowering=False, debug=False,
                   num_devices=N_CORES)
    rin = nc.dram_tensor("rin", [D, C], f32, kind="ExternalInput").ap()
    w1i = nc.dram_tensor("w1i", [128, DCH * E * S], f32, kind="ExternalInput").ap()
    w2i = nc.dram_tensor("w2i", [128, E * D], f32, kind="ExternalInput").ap()
    b1i = nc.dram_tensor("b1i", [128, E], f32, kind="ExternalInput").ap()
    b2i = nc.dram_tensor("b2i", [128, E * DCH], f32, kind="ExternalInput").ap()
    oout = nc.dram_tensor("oout", [D, C], f32, kind="ExternalOutput").ap()

    with tile.TileContext(nc) as tc, ExitStack() as ctx:
        wpool = ctx.enter_context(tc.tile_pool(name="weights", bufs=1))
        xpool = ctx.enter_context(tc.tile_pool(name="xin", bufs=4))
        hpool = ctx.enter_context(tc.tile_pool(name="h", bufs=3))
        opool = ctx.enter_context(tc.tile_pool(name="osb", bufs=3))
        hps = ctx.enter_context(tc.tile_pool(name="hps", bufs=2, space="PSUM"))
        ops = ctx.enter_context(tc.tile_pool(name="ops", bufs=4, space="PSUM"))

        w1s = wpool.tile([128, DCH * E * S], f32)
        nc.sync.dma_start(w1s[:], w1i)
        w2s = wpool.tile([128, E * D], f32)
        nc.sync.dma_start(w2s[:], w2i)
        b1s = wpool.tile([128, E], f32)
        nc.sync.dma_start(b1s[:], b1i)
        b2s = wpool.tile([128, E * DCH], f32)
        nc.sync.dma_start(b2s[:], b2i)

        def body():
            for (e, t0, n) in blocks:
                xt = xpool.tile([128, DCH * n], f32, tag="xt")
                src = rin[:, t0:t0 + n].rearrange("(c p) t -> p c t", p=128)
                dst = xt[:].rearrange("p (c t) -> p c t", c=DCH)
                nc.sync.dma_start(dst, src)

                hp = hps.tile([128, n], f32, tag="hp")
                for c in range(DCH):
                    nc.tensor.matmul(
                        hp[:],
                        w1s[:, (e * DCH + c) * S:(e * DCH + c + 1) * S],
                        xt[:, c * n:(c + 1) * n],
                        start=(c == 0), stop=(c == DCH - 1),
                    )
                hs = hpool.tile([128, n], f32, tag="hs")
                nc.scalar.activation(hs[:], hp[:], Relu, bias=b1s[:, e:e + 1])

                ot = opool.tile([128, DCH * n], f32, tag="ot")
                for m in range(DCH):
                    op = ops.tile([128, n], f32, tag="op")
                    nc.tensor.matmul(
                        op[:],
                        w2s[:, e * D + m * 128:e * D + (m + 1) * 128],
                        hs[:],
                        start=True, stop=True,
                    )
                    nc.vector.tensor_scalar_add(
                        ot[:, m * n:(m + 1) * n], op[:],
                        b2s[:, e * DCH + m:e * DCH + m + 1],
                    )

                osrc = ot[:].rearrange("p (c t) -> p c t", c=DCH)
                odst = oout[:, t0:t0 + n].rearrange("(c p) t -> p c t", p=128)
                nc.sync.dma_start(odst, osrc)

        if loop_n == 1:
            body()
        else:
            with tc.For_i(0, loop_n, 1):
                body()

    nc.compile()
    _PROGRAM_CACHE[key] = nc
    return nc


def _plan(yi, E):
    """Token permutation plan: per-core per-expert counts identical across
    cores, so one program serves all cores."""
    order = np.argsort(yi, kind="stable")
    counts = np.bincount(yi, minlength=E)
    c = -(-counts // N_CORES)  # ceil
    C = int(c.sum())
    perm = np.zeros((N_CORES, C), dtype=np.int64)
    valid = np.zeros((N_CORES, C), dtype=bool)
    blocks = []
    off = 0
    col = 0
    for e in range(E):
        n_e = int(counts[e])
        ce = int(c[e])
        if ce == 0:
            continue
        seg = order[off:off + n_e]
        padded = np.empty(N_CORES * ce, dtype=np.int64)
        padded[:n_e] = seg
        padded[n_e:] = seg[-1] if n_e > 0 else 0
        v = np.zeros(N_CORES * ce, dtype=bool)
        v[:n_e] = True
        perm[:, col:col + ce] = padded.reshape(N_CORES, ce)
        valid[:, col:col + ce] = v.reshape(N_CORES, ce)
        for t0 in range(0, ce, TBLK):
            blocks.append((e, col + t0, min(TBLK, ce - t0)))
        off += n_e
        col += ce
    assert col == C
    return blocks, perm, valid, C


def _prep_inputs(x, yi, z, W1, b1, W2, b2):
    """Host-side routing + layout prep shared by kernel() and the timing
    harness.  Returns (blocks, perm, valid, C, in_maps)."""
    B, D = x.shape
    E, _, S = W1.shape
    DCH = D // 128

    blocks, perm, valid, C = _plan(yi, E)

    r = np.maximum(x, 0.0)
    rin = np.ascontiguousarray(
        r[perm.reshape(-1)].reshape(N_CORES, C, D).transpose(0, 2, 1))

    w1i = np.ascontiguousarray(
        W1.reshape(E, DCH, 128, S).transpose(2, 0, 1, 3).reshape(128, E * DCH * S))
    w2i = np.ascontiguousarray(W2.transpose(1, 0, 2).reshape(128, E * D))
    b1i = np.ascontiguousarray(b1.T)  # [S=128, E]
    b2i = np.ascontiguousarray(
        b2.reshape(E, DCH, 128).transpose(2, 0, 1).reshape(128, E * DCH))

    in_maps = [
        {"rin": rin[m], "w1i": w1i, "w2i": w2i, "b1i": b1i, "b2i": b2i}
        for m in range(N_CORES)
    ]
    return blocks, perm, valid, C, in_maps


def kernel(x, y_index, y_hard, z, W1, b1, W2, b2):
    import sys
    if "/opt/trn_rl_repo" not in sys.path:
        sys.path.insert(0, "/opt/trn_rl_repo")
    from concourse import bass_utils

    x = np.ascontiguousarray(np.asarray(x, dtype=np.float32))
    z = np.asarray(z, dtype=np.float32)
    W1 = np.asarray(W1, dtype=np.float32)
    b1 = np.asarray(b1, dtype=np.float32)
    W2 = np.asarray(W2, dtype=np.float32)
    b2 = np.asarray(b2, dtype=np.float32)
    yi = np.asarray(y_index).reshape(-1).astype(np.int64)

    B, D = x.shape
    E, _, S = W1.shape

    blocks, perm, valid, C, in_maps = _prep_inputs(x, yi, z, W1, b1, W2, b2)
    nc = _get_program(blocks, C, D, S, E)

    res = bass_utils.run_bass_kernel_spmd(nc, in_maps,
                                          core_ids=list(range(N_CORES)))

    # Gather: oout[m] is [D, C]; o for padded slot (m, t) lives at [:, t].
    o_perm = np.stack([res.results[m]["oout"] for m in range(N_CORES)], axis=0)
    o_perm = o_perm.transpose(0, 2, 1).reshape(N_CORES * C, D)

    vflat = valid.reshape(-1)
    dest = perm.reshape(-1)[vflat]
    out = x.copy()
    out[dest] = x[dest] + z[dest] * o_perm[vflat]
    return out
